# revision 1
# baseline (speedup 1.0000x reference)
"""ChannelAttention (LKA3D) Trainium2 Bass kernel, v2.

Problem: B=4, N=16384, C=384, heads=4, head_dim=96.
Reference: qkv = x @ W_qkv.T; per head q,k,v transposed to (d, N);
q,k L2-normalized over N; attn = softmax((q@k.T)*temp, axis=-1);
out = (attn @ v) reassembled to (B,N,C) @ W_out.T + b_out.

Key restructure vs v1: channel attention only needs the d x d gram and
per-channel norms, all bilinear in x. With S = X.T @ X (384x384):
  G_h      = Wq_h.T S Wk_h          (per-head 96x96 logits)
  ||q_c||^2 = diag(Wq.T S Wq),  ||k_d||^2 = diag(Wk.T S Wk)
and the output path folds completely:
  out = V P = X (Wv.T P) = X M,  M = Wv.T (blockdiag(attn).T Wo)
So the N-sized work per batch is only S (= half the qkv projection) and
X @ M (= the out projection); everything else is 384x384.

Sharding: tokens split across 8 cores (2048/batch/core). S is summed
over tokens, so each core computes partial G/norm stats from its local
S; one 147KB/batch AllReduce (same as v1) yields full stats; attention
weights are computed redundantly per core; X @ M uses local tokens.

Layouts: x loaded twice -- token-major (S build lhsT/rhs) and
channel-major (X@M lhsT). d/c axes run in 3 chunks of 128 partitions
(not 4 heads of 96) so every matmul uses the full PE array.
Matmuls in float32r (full rate at N>=256); the small N=96 G/psc
matmuls use bf16 operands (1 cycle/row).
"""

import numpy as np
import concourse.bacc as bacc
import concourse.mybir as mybir
from concourse import tile
from concourse.bass_utils import run_bass_kernel_spmd

F32 = mybir.dt.float32
F32R = mybir.dt.float32r
BF16 = mybir.dt.bfloat16
ALU = mybir.AluOpType
ACTF = mybir.ActivationFunctionType

B = 4
C = 384
NHEADS = 4
DH = 96
NCORES = 8
NFULL = 16384
NL = NFULL // NCORES   # 2048 tokens per core per batch
NT = NL // 128         # 16 token tiles per batch
NCH = NL // 512        # 4 512-token chunks per batch
STATS_LEN = 96 * 384 + 768  # G block + [sum q^2 | sum k^2] rows



def build_nc(loop_n=1, use_collective=True):
    nc = bacc.Bacc(None, target_bir_lowering=False, debug=False)
    XTM = nc.dram_tensor("xtm", [B, NL, C], F32R, kind="ExternalInput")
    XCM = nc.dram_tensor("xcm", [B, 3, 128, NL], BF16, kind="ExternalInput")
    WQ = nc.dram_tensor("wq", [3, 128, C], F32R, kind="ExternalInput")
    WK = nc.dram_tensor("wk", [3, 128, C], F32R, kind="ExternalInput")
    WV = nc.dram_tensor("wv", [96, 4, C], F32R, kind="ExternalInput")
    WO = nc.dram_tensor("wo", [NHEADS, DH, C], F32R, kind="ExternalInput")
    BIAS = nc.dram_tensor("bias", [3, 128], F32, kind="ExternalInput")
    TEMP = nc.dram_tensor("temp", [NHEADS], F32, kind="ExternalInput")
    OUT = nc.dram_tensor("out", [B, 3, 128, NL], F32, kind="ExternalOutput")
    stats_in = [
        nc.dram_tensor(f"stats_in{b}", [STATS_LEN], F32) for b in range(B)
    ]
    stats_out = [
        nc.dram_tensor(f"stats_out{b}", [STATS_LEN], F32, addr_space="Shared")
        for b in range(B)
    ]

    def g_view(t):
        return t.ap()[0 : 96 * 384].rearrange("(p f) -> p f", p=96)

    def sq_view(t):
        return t.ap()[96 * 384 : STATS_LEN][None, :]

    with tile.TileContext(nc) as tc:
        with (
            tc.tile_pool(name="wpool", bufs=1) as wpool,
            tc.tile_pool(name="xtmp", bufs=10) as xtmp,
            tc.tile_pool(name="xcmp", bufs=2) as xcmp,
            tc.tile_pool(name="spool", bufs=2) as spool,
            tc.tile_pool(name="apool", bufs=2) as apool,
            tc.tile_pool(name="gpool", bufs=2) as gpool,
            tc.tile_pool(name="p2pool", bufs=2) as p2pool,
            tc.tile_pool(name="opool", bufs=6) as opool,
            tc.tile_pool(name="pS", bufs=1, space="PSUM") as pS,
            tc.tile_pool(name="pAG", bufs=2, space="PSUM") as pAG,
            tc.tile_pool(name="pO", bufs=3, space="PSUM") as pO,
        ):
            xt_pre = {}
            for _ch in range(2 * NCH):
                _xt = xtmp.tile([128, 2, C], F32R, name="xtm", tag="xtm")
                nc.sync.dma_start(
                    out=_xt[:, :, :],
                    in_=XTM[0, _ch * 256 : (_ch + 1) * 256, :].rearrange(
                        "(t p) c -> p t c", p=128
                    ),
                )
                xt_pre[_ch] = _xt
            wq_sb = wpool.tile([128, 3, C], F32R, name="wq", tag="wq")
            wk_sb = wpool.tile([128, 3, C], F32R, name="wk", tag="wk")
            wv_sb = wpool.tile([96, 4, C], F32R, name="wv", tag="wv")
            nc.sync.dma_start(out=wq_sb[:, :, :], in_=WQ.ap().rearrange("a p f -> p a f"))
            nc.sync.dma_start(out=wk_sb[:, :, :], in_=WK.ap().rearrange("a p f -> p a f"))
            nc.sync.dma_start(out=wv_sb[:, :, :], in_=WV.ap())
            wq_bf = wpool.tile([128, 3, C], BF16, name="wqbf", tag="wqbf")
            wk_bf = wpool.tile([128, 3, C], BF16, name="wkbf", tag="wkbf")
            nc.vector.tensor_copy(wq_bf[:, :, :], wq_sb[:, :, :])
            nc.vector.tensor_copy(wk_bf[:, :, :], wk_sb[:, :, :])
            wo_sb = []
            for h in range(NHEADS):
                t = wpool.tile([DH, C], F32R, name=f"wo{h}", tag=f"wo{h}")
                nc.sync.dma_start(out=t[:, :], in_=WO[h, :, :])
                wo_sb.append(t)
            bias_ch = wpool.tile([128, 3], F32, name="bias", tag="bias")
            nc.sync.dma_start(out=bias_ch[:, :], in_=BIAS.ap().rearrange("a p -> p a"))
            temp_sb = wpool.tile([1, NHEADS], F32, name="temp", tag="temp")
            nc.sync.dma_start(out=temp_sb[:, :], in_=TEMP.ap()[None, :])
            ones = wpool.tile([128, 1], F32R, name="ones", tag="ones")
            nc.vector.memset(ones[:, :].bitcast(F32), 1.0)

            state = {}

            def p1_s(b):
                xcm = xcmp.tile([128, 3, NL], BF16, name="xcm", tag="xcm")
                # S = X.T X accumulated over 16 token tiles into 3 PSUM banks
                sacc = [
                    pS.tile([128, C], F32, name=f"s{i}", tag=f"ps{i}") for i in range(3)
                ]
                for ch in range(2 * NCH):
                    if b == 0:
                        xt = xt_pre.pop(ch)
                    else:
                        xt = xtmp.tile([128, 2, C], F32R, name="xtm", tag="xtm")
                        nc.sync.dma_start(
                            out=xt[:, :, :],
                            in_=XTM[b, ch * 256 : (ch + 1) * 256, :].rearrange(
                                "(t p) c -> p t c", p=128
                            ),
                        )
                    for t in range(2):
                        nt = ch * 2 + t
                        for cc in range(3):
                            nc.tensor.matmul(
                                sacc[cc][:, :],
                                xt[:, t, cc * 128 : (cc + 1) * 128],
                                xt[:, t, :],
                                start=(nt == 0),
                                stop=(nt == NT - 1),
                            )
                # channel-major x for this batch's phase2 (consumed only
                # after the collective) -- after the xtm loads on sync so it
                # never delays the S build or the stats path.
                nc.sync.dma_start(
                    out=xcm[:, :, :], in_=XCM[b, :, :, :].rearrange("a p n -> p a n")
                )
                s_sb = spool.tile([128, 3, C], F32R, name="s", tag="s")
                for cc in range(3):
                    nc.scalar.copy(s_sb[:, cc, :], sacc[cc][:, :])
                state[b] = (xcm, s_sb)

            def p1_stats(b):
                xcm, s_sb = state[b]
                # A = S @ Wk_st, A' = S @ Wq_st (row-chunk lhsT via S symmetry)
                ab_sb = []
                for w_sb, nm, want_bf in ((wk_sb, "a", True), (wq_sb, "ap", False)):
                    a_sb = apool.tile([128, 3, C], F32R, name=nm, tag=nm)
                    a_bf = (
                        apool.tile([128, 3, C], BF16, name=nm + "b", tag=nm + "b")
                        if want_bf
                        else None
                    )
                    for co in range(3):
                        pa = pAG.tile([128, C], F32, name="pa", tag="pa")
                        for e in range(3):
                            nc.tensor.matmul(
                                pa[:, :],
                                s_sb[:, e, co * 128 : (co + 1) * 128],
                                w_sb[:, e, :],
                                start=(e == 0),
                                stop=(e == 2),
                            )
                        nc.scalar.copy(a_sb[:, co, :], pa[:, :])
                        if want_bf:
                            nc.vector.tensor_copy(a_bf[:, co, :], pa[:, :])
                    ab_sb.append((a_sb, a_bf))
                (a_sb, a_bf), (ap_sb, _) = ab_sb
                # G_h = Wq_h.T A_h (bf16 operands, 96-col outputs)
                gacc = gpool.tile([96, C], F32, name="gacc", tag="gacc")
                for h in range(NHEADS):
                    hs = slice(h * 96, (h + 1) * 96)
                    pg = pAG.tile([96, 96], F32, name="pg", tag="pa")
                    for cc in range(3):
                        nc.tensor.matmul(
                            pg[:, :],
                            wq_bf[:, cc, hs],
                            a_bf[:, cc, hs],
                            start=(cc == 0),
                            stop=(cc == 2),
                        )
                    nc.vector.tensor_copy(gacc[:, hs], pg[:, :])
                # diag(Wq.T S Wq) / diag(Wk.T S Wk): elementwise W*A summed
                # over the 3 c-chunks on DVE, partition-reduced by ones matmul
                sqrow = gpool.tile([1, 768], F32, name="sqrow", tag="sqrow")
                for j, (w_sb, asb) in enumerate(((wq_sb, ap_sb), (wk_sb, a_sb))):
                    prod = gpool.tile([128, C], F32R, name="prod", tag="prod")
                    with nc.allow_low_precision(reason="f32r matmul operand"):
                        nc.vector.tensor_tensor(
                            prod[:, :],
                            w_sb[:, 0, :].bitcast(F32),
                            asb[:, 0, :].bitcast(F32),
                            ALU.mult,
                        )
                    for cc in (1, 2):
                        tmp = gpool.tile([128, C], F32, name="ptmp", tag="ptmp")
                        nc.vector.tensor_tensor(
                            tmp[:, :],
                            w_sb[:, cc, :].bitcast(F32),
                            asb[:, cc, :].bitcast(F32),
                            ALU.mult,
                        )
                        with nc.allow_low_precision(reason="f32r matmul operand"):
                            nc.vector.tensor_add(
                                prod[:, :], prod[:, :].bitcast(F32), tmp[:, :]
                            )
                    pd = pAG.tile([1, C], F32, name="pd", tag="pa")
                    nc.tensor.matmul(pd[0:1, :], ones[:, :], prod[:, :],
                                     start=True, stop=True)
                    nc.vector.tensor_copy(sqrow[0:1, j * C : (j + 1) * C], pd[0:1, :])
                nc.scalar.dma_start(out=g_view(stats_in[b]), in_=gacc[:, :])
                nc.scalar.dma_start(out=sq_view(stats_in[b]), in_=sqrow[0:1, :])
                if use_collective:
                    nc.gpsimd.collective_compute(
                        "AllReduce",
                        ALU.add,
                        replica_groups=[list(range(NCORES))],
                        ins=[stats_in[b].ap().opt()],
                        outs=[stats_out[b].ap().opt()],
                    )
                stats_src = stats_out[b] if use_collective else stats_in[b]
                g_all = gpool.tile([96, C], F32, name="gall", tag="gall")
                sq_all = gpool.tile([1, 768], F32, name="sqall", tag="sqall")
                nc.scalar.dma_start(out=g_all[:, :], in_=g_view(stats_src))
                nc.scalar.dma_start(out=sq_all[0:1, :], in_=sq_view(stats_src))
                t768 = gpool.tile([1, 768], F32, name="t768", tag="t768")
                rsq = gpool.tile([1, 768], F32R, name="rsq", tag="rsq")
                nc.vector.tensor_scalar_max(t768[0:1, :], sq_all[0:1, :], 1e-24)
                nc.scalar.sqrt(t768[0:1, :], t768[0:1, :])
                with nc.allow_low_precision(reason="f32r operands for matmul"):
                    nc.vector.reciprocal(rsq[0:1, :], t768[0:1, :])
                state[b] = (xcm, (g_all, rsq))

            def p2_pm(b):
                xcm, (g_all, rsq) = state[b]
                p_sb = p2pool.tile([96, 4, C], F32R, name="p", tag="p")
                for h in range(NHEADS):
                    hs = slice(h * 96, (h + 1) * 96)
                    rkt = gpool.tile([1, 96], F32R, name="rkt", tag="rkt")
                    nc.vector.tensor_scalar_mul(
                        rkt[0:1, :],
                        rsq[0:1, 384 + h * 96 : 384 + (h + 1) * 96],
                        temp_sb[0:1, h : h + 1],
                    )
                    psc = pAG.tile([96, 96], F32, name="psc", tag="pa")
                    nc.tensor.matmul(
                        psc[:, 0:96],
                        rsq[0:1, hs],
                        rkt[0:1, :],
                        start=True,
                        stop=True,
                    )
                    logit = p2pool.tile([96, 96], F32, name="logit", tag="logit")
                    nc.vector.tensor_tensor(
                        logit[:, :], g_all[:, hs], psc[:, 0:96], ALU.mult
                    )
                    expt = p2pool.tile([96, 96], F32, name="exp", tag="exp")
                    den = p2pool.tile([96, 1], F32, name="den", tag="den")
                    nc.scalar.activation(
                        expt[:, :], logit[:, :], ACTF.Exp, accum_out=den[:, 0:1]
                    )
                    denr = p2pool.tile([96, 1], F32, name="denr", tag="denr")
                    nc.vector.reciprocal(denr[:, 0:1], den[:, 0:1])
                    attn = p2pool.tile([96, 96], F32R, name="attn", tag="attn")
                    nc.vector.tensor_scalar_mul(attn[:, :], expt[:, :], denr[:, 0:1])
                    pp = pAG.tile([96, C], F32, name="pp", tag="pa")
                    nc.tensor.matmul(
                        pp[:, :], attn[:, :], wo_sb[h][:, :], start=True, stop=True
                    )
                    nc.scalar.copy(p_sb[:, h, :], pp[:, :])
                # M = Wv.T P  (contract the 4 heads' 96-row blocks per c-chunk)
                m_sb = p2pool.tile([128, 3, C], BF16, name="m", tag="m")
                for co in range(3):
                    pm = pAG.tile([128, C], F32, name="pm", tag="pa")
                    for h in range(NHEADS):
                        nc.tensor.matmul(
                            pm[:, :],
                            wv_sb[:, h, co * 128 : (co + 1) * 128],
                            p_sb[:, h, :],
                            start=(h == 0),
                            stop=(h == 3),
                        )
                    nc.scalar.copy(m_sb[:, co, :], pm[:, :])
                state[b] = (xcm, m_sb)

            def p2_out(b):
                xcm, m_sb = state.pop(b)
                # out.T = M.T X (channel-major): stationary m-slice reused
                # across the 4 token chunks; 512-wide bf16 streams
                for ms in range(3):
                    for half in range(2):
                        pos = [
                            pO.tile([128, 512], F32, name=f"po{tc}", tag="po")
                            for tc in range(2)
                        ]
                        for cc in range(3):
                            for j in range(2):
                                tc = half * 2 + j
                                nc.tensor.matmul(
                                    pos[j][:, :],
                                    m_sb[:, cc, ms * 128 : (ms + 1) * 128],
                                    xcm[:, cc, tc * 512 : (tc + 1) * 512],
                                    start=(cc == 0),
                                    stop=(cc == 2),
                                )
                        for j in range(2):
                            tc = half * 2 + j
                            obuf = opool.tile([128, 512], F32, name="osb", tag="osb")
                            nc.vector.tensor_scalar_add(
                                obuf[:, :], pos[j][:, :], bias_ch[:, ms : ms + 1]
                            )
                            eng = (nc.sync, nc.gpsimd, nc.gpsimd)[ms]
                            eng.dma_start(
                                out=OUT[b, ms, :, tc * 512 : (tc + 1) * 512],
                                in_=obuf[:, :],
                            )

            def body():
                p1_s(0)
                p1_stats(0)
                p1_s(1)
                p1_stats(1)
                p2_pm(0)
                p2_out(0)
                p1_s(2)
                p2_pm(1)
                p1_stats(2)
                p2_out(1)
                p1_s(3)
                p2_pm(2)
                p1_stats(3)
                p2_out(2)
                p2_pm(3)
                p2_out(3)
                if not use_collective:
                    # traffic-parity stand-in for the AllReduce output leg,
                    # off the critical path
                    for b in range(B):
                        nc.gpsimd.dma_start(
                            out=stats_out[b].ap(), in_=stats_in[b].ap()
                        )

            if loop_n > 1:
                _eng = mybir.EngineType
                with tc.For_i(
                    0, loop_n, 1, staggered_reset=True,
                    hint_engines=(_eng.PE, _eng.DVE, _eng.Activation, _eng.SP, _eng.Pool),
                ):
                    body()
            else:
                body()

    nc.compile()
    return nc


_NC_CACHE = {}


def get_nc(loop_n=1, use_collective=True):
    key = (loop_n, use_collective)
    if key not in _NC_CACHE:
        _NC_CACHE[key] = build_nc(loop_n, use_collective)
    return _NC_CACHE[key]


def prep_in_maps(x, W_qkv, temperature_ch, W_out, b_out):
    import ml_dtypes
    x = np.asarray(x, np.float32)
    W_qkv = np.asarray(W_qkv, np.float32)
    W_out = np.asarray(W_out, np.float32)
    b_out = np.asarray(b_out, np.float32)
    temp = np.asarray(temperature_ch, np.float32).reshape(-1)
    xbf = x.astype(ml_dtypes.bfloat16)
    xt = np.ascontiguousarray(xbf.transpose(0, 2, 1)).reshape(B, 3, 128, NFULL)
    wq = np.ascontiguousarray(W_qkv[0:384].T).reshape(3, 128, C)
    wk = np.ascontiguousarray(W_qkv[384:768].T).reshape(3, 128, C)
    wv = np.ascontiguousarray(W_qkv[768:1152].reshape(4, 96, C).transpose(1, 0, 2))
    wo = np.stack(
        [np.ascontiguousarray(W_out[:, h * 96 : (h + 1) * 96].T) for h in range(4)]
    )
    shared = {"wq": wq, "wk": wk, "wv": wv, "wo": wo,
              "bias": np.ascontiguousarray(b_out.reshape(3, 128)), "temp": temp}
    return [
        dict(
            shared,
            xtm=np.ascontiguousarray(x[:, i * NL : (i + 1) * NL, :]),
            xcm=np.ascontiguousarray(xt[:, :, :, i * NL : (i + 1) * NL]),
        )
        for i in range(NCORES)
    ]


def kernel(**inputs):
    nc = get_nc(1)
    in_maps = prep_in_maps(
        inputs["x"],
        inputs["W_qkv"],
        inputs["temperature_ch"],
        inputs["W_out"],
        inputs["b_out"],
    )
    res = run_bass_kernel_spmd(nc, in_maps, core_ids=list(range(NCORES)))
    out = np.empty((B, NFULL, C), np.float32)
    for i in range(NCORES):
        # device output is channel-major [B, 3, 128, NL]
        r = res.results[i]["out"]
        out[:, i * NL : (i + 1) * NL, :] = r.transpose(0, 3, 1, 2).reshape(B, NL, C)
    return out



# revision 8
# speedup vs baseline: 1.7153x; 1.7153x over previous
"""ChannelAttention (LKA3D) Trainium2 Bass kernel, v3.

Problem: B=4, N=16384, C=384, heads=4, head_dim=96.
Reference: qkv = x @ W_qkv.T; per head q,k,v transposed to (d, N);
q,k L2-normalized over N; attn = softmax((q@k.T)*temp, axis=-1);
out = (attn @ v) reassembled to (B,N,C) @ W_out.T + b_out.

Restructure (from v2): channel attention only needs the d x d gram and
per-channel norms, all bilinear in x. With S = X.T @ X (384x384):
  G_h       = Wq_h.T S Wk_h          (per-head 96x96 logits)
  ||q_c||^2 = diag(Wq.T S Wq),  ||k_d||^2 = diag(Wk.T S Wk)
and the output path folds completely:
  out = V P = X (Wv.T P) = X M,  M = Wv.T (blockdiag(attn).T Wo)
So the N-sized work per core is only S and X @ M; everything else is
384x384.

v3 sharding: core c handles batch c//2, token half c%2 (8192 tokens).
Each core computes partial stats from its half; a 2-party AllReduce per
pair [[0,1],[2,3],[4,5],[6,7]] of the 147KB stats block completes them.
The stats/attention small-op chain thus runs ONCE per core (v2 ran it
4x), and the qkv/attn/M matmuls shrink 4x per core.

Dtypes (validated in numpy against the reference, rel err 3.7e-3 vs
2e-2 budget): S built from fp8e4 x with DoubleRow matmuls (2 rows/cyc);
A = S@W in f32r, downstream stats in bf16; out phase bf16 X / bf16 M;
output downloaded as bf16 and upcast on host (halves output DMA).
"""

import numpy as np
import concourse.bacc as bacc
import concourse.mybir as mybir
from concourse import tile
from concourse.bass_utils import run_bass_kernel_spmd

F32 = mybir.dt.float32
F32R = mybir.dt.float32r
BF16 = mybir.dt.bfloat16
F8 = mybir.dt.float8e4
ALU = mybir.AluOpType
ACTF = mybir.ActivationFunctionType
DR = mybir.MatmulPerfMode.DoubleRow

B = 4
C = 384
NHEADS = 4
DH = 96
NCORES = 8
NFULL = 16384
NL = NFULL // 2        # 8192 tokens per core (one half of one batch)
NK = NL // 256         # 32 fp8 double-row pair tiles
NTC = NL // 512        # 16 out-phase token chunks
STATS_ROWS = 98        # 96 G rows + 2 sq rows


def build_nc(loop_n=1, use_collective=True):
    nc = bacc.Bacc(None, target_bir_lowering=False, debug=False)
    XT8 = nc.dram_tensor("xt8", [NK, 128, 2, C], F8, kind="ExternalInput")
    XCM = nc.dram_tensor("xcm", [3, 128, NL], BF16, kind="ExternalInput")
    WQ = nc.dram_tensor("wq", [3, 128, C], F32R, kind="ExternalInput")
    WK = nc.dram_tensor("wk", [3, 128, C], F32R, kind="ExternalInput")
    WV = nc.dram_tensor("wv", [96, 4, C], F32R, kind="ExternalInput")
    WO = nc.dram_tensor("wo", [NHEADS, DH, C], F32R, kind="ExternalInput")
    BIAS = nc.dram_tensor("bias", [3, 128], F32, kind="ExternalInput")
    TEMP = nc.dram_tensor("temp", [NHEADS], F32, kind="ExternalInput")
    OUT = nc.dram_tensor("out", [3, 128, NL], BF16, kind="ExternalOutput")
    STATS_IN = nc.dram_tensor("stats_in", [STATS_ROWS * C], F32)
    STATS_OUT = nc.dram_tensor("stats_out", [STATS_ROWS * C], F32)

    def stats_view(t):
        return t.ap().rearrange("(p f) -> p f", p=STATS_ROWS)

    with tile.TileContext(nc) as tc:
        with (
            tc.tile_pool(name="wpool", bufs=1) as wpool,
            tc.tile_pool(name="xpool", bufs=1) as xpool,
            tc.tile_pool(name="spool", bufs=2) as spool,
            tc.tile_pool(name="apool", bufs=2) as apool,
            tc.tile_pool(name="gpool", bufs=2) as gpool,
            tc.tile_pool(name="p2pool", bufs=2) as p2pool,
            tc.tile_pool(name="opool", bufs=2) as opool,
            tc.tile_pool(name="pS", bufs=1, space="PSUM") as pS,
            tc.tile_pool(name="pA", bufs=2, space="PSUM") as pA,
            tc.tile_pool(name="pO", bufs=3, space="PSUM") as pO,
        ):
            # ---- weights: loaded once (resident across loop iterations) ----
            wq_sb = wpool.tile([128, 3, C], F32R, name="wq", tag="wq")
            wk_sb = wpool.tile([128, 3, C], F32R, name="wk", tag="wk")
            wv_sb = wpool.tile([96, 4, C], F32R, name="wv", tag="wv")
            wo_sb = wpool.tile([96, NHEADS, C], F32R, name="wo", tag="wo")
            nc.scalar.dma_start(out=wq_sb[:, :, :], in_=WQ.ap().rearrange("a p f -> p a f"))
            nc.scalar.dma_start(out=wk_sb[:, :, :], in_=WK.ap().rearrange("a p f -> p a f"))
            nc.scalar.dma_start(out=wv_sb[:, :, :], in_=WV.ap())
            nc.scalar.dma_start(out=wo_sb[:, :, :], in_=WO.ap().rearrange("h p f -> p h f"))
            bias_ch = wpool.tile([128, 3], F32, name="bias", tag="bias")
            nc.scalar.dma_start(out=bias_ch[:, :], in_=BIAS.ap().rearrange("a p -> p a"))
            temp_sb = wpool.tile([1, NHEADS], F32, name="temp", tag="temp")
            nc.scalar.dma_start(out=temp_sb[:, :], in_=TEMP.ap()[None, :])
            ones = wpool.tile([128, 1], BF16, name="ones", tag="ones")
            nc.vector.memset(ones[:, :], 1.0)
            wq_bf = wpool.tile([128, 3, C], BF16, name="wqbf", tag="wqbf")
            wk_bf = wpool.tile([128, 3, C], BF16, name="wkbf", tag="wkbf")
            nc.vector.tensor_copy(wq_bf[:, :, :], wq_sb[:, :, :].bitcast(F32))
            nc.vector.tensor_copy(wk_bf[:, :, :], wk_sb[:, :, :].bitcast(F32))

            def body():
                # ---- input loads (all inside body for loop-timing parity) --
                xt8 = xpool.tile([128, NK, 2, C], F8, name="xt8", tag="xt8")
                for i in range(2):
                    nc.sync.dma_start(
                        out=xt8[:, i * 16 : (i + 1) * 16, :, :],
                        in_=XT8.ap()[i * 16 : (i + 1) * 16].rearrange(
                            "k p t c -> p k t c"
                        ),
                    )
                xcm = xpool.tile([128, 3, NL], BF16, name="xcm", tag="xcm")
                for cc in range(3):
                    nc.gpsimd.dma_start(out=xcm[:, cc, :], in_=XCM.ap()[cc])

                # ---- S = X^T X via fp8 DoubleRow (256 tokens/matmul) -------
                sacc = [
                    pS.tile([128, C], F32, name=f"s{i}", tag=f"ps{i}")
                    for i in range(3)
                ]
                for k in range(NK):
                    for cc in range(3):
                        nc.tensor.matmul(
                            sacc[cc][:, :],
                            xt8[:, k, :, cc * 128 : (cc + 1) * 128],
                            xt8[:, k, :, :],
                            start=(k == 0),
                            stop=(k == NK - 1),
                            perf_mode=DR,
                        )
                s_sb = spool.tile([128, 3, C], F32R, name="s", tag="s")
                for cc in range(3):
                    nc.scalar.copy(s_sb[:, cc, :], sacc[cc][:, :])

                # ---- A = S @ Wk, A' = S @ Wq (bf16 results) ----------------
                ab = {}
                for w_sb, nm in ((wk_sb, "a"), (wq_sb, "ap")):
                    a_bf = apool.tile([128, 3, C], BF16, name=nm, tag=nm)
                    for co in range(3):
                        pa = pA.tile([128, C], F32, name="pa", tag="pa")
                        for e in range(3):
                            nc.tensor.matmul(
                                pa[:, :],
                                s_sb[:, e, co * 128 : (co + 1) * 128],
                                w_sb[:, e, :],
                                start=(e == 0),
                                stop=(e == 2),
                            )
                        eng = nc.scalar if co != 1 else None
                        if eng is not None:
                            eng.copy(a_bf[:, co, :], pa[:, :])
                        else:
                            nc.vector.tensor_copy(a_bf[:, co, :], pa[:, :])
                    ab[nm] = a_bf
                a_bf, ap_bf = ab["a"], ab["ap"]

                # ---- stats block: G rows 0:96, sq rows 96:98 ---------------
                gacc = gpool.tile([96, C], F32, name="gacc", tag="gacc")
                sqrow = gpool.tile([1, 768], F32, name="sqrow", tag="sqrow")
                for h in range(NHEADS):
                    hs = slice(h * DH, (h + 1) * DH)
                    pg = pA.tile([96, 96], F32, name="pg", tag="pa")
                    for cc in range(3):
                        nc.tensor.matmul(
                            pg[:, :],
                            wq_bf[:, cc, hs],
                            a_bf[:, cc, hs],
                            start=(cc == 0),
                            stop=(cc == 2),
                        )
                    nc.vector.tensor_copy(gacc[:, hs], pg[:, :])
                # diag(W.T S W) via elementwise W*A (Pool) + ones-matmul
                for j, (w_bf, asb) in enumerate(
                    ((wq_bf, ap_bf), (wk_bf, a_bf))
                ):
                    pd = pA.tile([1, C], F32, name="pd", tag="pa")
                    for cc in range(3):
                        prod = gpool.tile(
                            [128, C], BF16, name=f"prod{cc}", tag=f"prod{cc}"
                        )
                        nc.gpsimd.tensor_tensor(
                            prod[:, :], w_bf[:, cc, :], asb[:, cc, :], ALU.mult
                        )
                        nc.tensor.matmul(
                            pd[0:1, :],
                            ones[:, :],
                            prod[:, :],
                            start=(cc == 0),
                            stop=(cc == 2),
                        )
                    nc.vector.tensor_copy(sqrow[0:1, j * C : (j + 1) * C], pd[0:1, :])

                nc.scalar.dma_start(
                    out=stats_view(STATS_IN)[0:96], in_=gacc[:, :]
                )
                nc.scalar.dma_start(
                    out=STATS_IN.ap()[96 * C : STATS_ROWS * C][None, :],
                    in_=sqrow[0:1, :],
                )
                if use_collective:
                    nc.gpsimd.collective_compute(
                        "AllReduce",
                        ALU.add,
                        replica_groups=[[2 * i, 2 * i + 1] for i in range(B)],
                        ins=[STATS_IN.ap().opt()],
                        outs=[STATS_OUT.ap().opt()],
                    )
                stats_src = STATS_OUT if use_collective else STATS_IN
                g_all = gpool.tile([96, C], F32, name="gall", tag="gall")
                sq_all = gpool.tile([1, 768], F32, name="sqall", tag="sqall")
                nc.scalar.dma_start(out=g_all[:, :], in_=stats_view(stats_src)[0:96])
                nc.gpsimd.dma_start(
                    out=sq_all[0:1, :],
                    in_=stats_src.ap()[96 * C : STATS_ROWS * C][None, :],
                )
                t768 = gpool.tile([1, 768], F32, name="t768", tag="t768")
                rsq = gpool.tile([1, 768], F32R, name="rsq", tag="rsq")
                nc.vector.tensor_scalar_max(t768[0:1, :], sq_all[0:1, :], 1e-24)
                nc.scalar.sqrt(t768[0:1, :], t768[0:1, :])
                with nc.allow_low_precision(reason="f32r operands for matmul"):
                    nc.vector.reciprocal(rsq[0:1, :], t768[0:1, :])

                # ---- attention weights + M = Wv.T blockdiag(attn).T Wo -----
                p_sb = p2pool.tile([96, NHEADS, C], F32R, name="p", tag="p")
                for h in range(NHEADS):
                    hs = slice(h * DH, (h + 1) * DH)
                    rkt = gpool.tile([1, 96], F32R, name="rkt", tag="rkt")
                    with nc.allow_low_precision(reason="f32r matmul operand"):
                        nc.vector.tensor_scalar_mul(
                            rkt[0:1, :],
                            rsq[0:1, 384 + h * DH : 384 + (h + 1) * DH].bitcast(F32),
                            temp_sb[0:1, h : h + 1],
                        )
                    psc = pA.tile([96, 96], F32, name="psc", tag="pa")
                    nc.tensor.matmul(
                        psc[:, 0:96], rsq[0:1, hs], rkt[0:1, :],
                        start=True, stop=True,
                    )
                    logit = p2pool.tile([96, 96], F32, name="logit", tag="logit")
                    nc.vector.tensor_tensor(
                        logit[:, :], g_all[:, hs], psc[:, 0:96], ALU.mult
                    )
                    expt = p2pool.tile([96, 96], F32, name="exp", tag="exp")
                    den = p2pool.tile([96, 1], F32, name="den", tag="den")
                    nc.scalar.activation(
                        expt[:, :], logit[:, :], ACTF.Exp, accum_out=den[:, 0:1]
                    )
                    denr = p2pool.tile([96, 1], F32, name="denr", tag="denr")
                    nc.vector.reciprocal(denr[:, 0:1], den[:, 0:1])
                    attn = p2pool.tile([96, 96], F32R, name="attn", tag="attn")
                    with nc.allow_low_precision(reason="f32r matmul operand"):
                        nc.vector.tensor_scalar_mul(
                            attn[:, :], expt[:, :], denr[:, 0:1]
                        )
                    pp = pA.tile([96, C], F32, name="pp", tag="pa")
                    nc.tensor.matmul(
                        pp[:, :], attn[:, :], wo_sb[:, h, :], start=True, stop=True
                    )
                    nc.scalar.copy(p_sb[:, h, :], pp[:, :])
                m_sb = p2pool.tile([128, 3, C], BF16, name="m", tag="m")
                for co in range(3):
                    pm = pA.tile([128, C], F32, name="pm", tag="pa")
                    for h in range(NHEADS):
                        nc.tensor.matmul(
                            pm[:, :],
                            wv_sb[:, h, co * 128 : (co + 1) * 128],
                            p_sb[:, h, :],
                            start=(h == 0),
                            stop=(h == NHEADS - 1),
                        )
                    nc.scalar.copy(m_sb[:, co, :], pm[:, :])

                # ---- out.T = M.T X (+bias), bf16 download ------------------
                for ms in range(3):
                    obuf = opool.tile([128, NL], BF16, name="osb", tag="osb")
                    for tcn in range(NTC):
                        po = pO.tile([128, 512], F32, name="po", tag="po")
                        for cc in range(3):
                            nc.tensor.matmul(
                                po[:, :],
                                m_sb[:, cc, ms * 128 : (ms + 1) * 128],
                                xcm[:, cc, tcn * 512 : (tcn + 1) * 512],
                                start=(cc == 0),
                                stop=(cc == 2),
                            )
                        nc.vector.tensor_scalar_add(
                            obuf[:, tcn * 512 : (tcn + 1) * 512],
                            po[:, :],
                            bias_ch[:, ms : ms + 1],
                        )
                    nc.sync.dma_start(out=OUT.ap()[ms], in_=obuf[:, :])

                if not use_collective:
                    # traffic-parity stand-in for the AllReduce leg
                    nc.gpsimd.dma_start(
                        out=STATS_OUT.ap(), in_=STATS_IN.ap()
                    )

            if loop_n > 1:
                _eng = mybir.EngineType
                with tc.For_i(
                    0, loop_n, 1, staggered_reset=True,
                    hint_engines=(_eng.PE, _eng.DVE, _eng.Activation, _eng.SP,
                                  _eng.Pool),
                ):
                    body()
            else:
                body()

    nc.compile()
    return nc


_NC_CACHE = {}


def get_nc(loop_n=1, use_collective=True):
    key = (loop_n, use_collective)
    if key not in _NC_CACHE:
        _NC_CACHE[key] = build_nc(loop_n, use_collective)
    return _NC_CACHE[key]


def prep_in_maps(x, W_qkv, temperature_ch, W_out, b_out):
    import ml_dtypes

    x = np.asarray(x, np.float32)
    W_qkv = np.asarray(W_qkv, np.float32)
    W_out = np.asarray(W_out, np.float32)
    b_out = np.asarray(b_out, np.float32)
    temp = np.asarray(temperature_ch, np.float32).reshape(-1)
    x8 = x.astype(ml_dtypes.float8_e4m3)
    xcm_full = np.ascontiguousarray(
        x.transpose(0, 2, 1).astype(ml_dtypes.bfloat16)
    )  # (B, C, N)
    wq = np.ascontiguousarray(W_qkv[0:C].T).reshape(3, 128, C)
    wk = np.ascontiguousarray(W_qkv[C : 2 * C].T).reshape(3, 128, C)
    wv = np.ascontiguousarray(
        W_qkv[2 * C : 3 * C].reshape(4, 96, C).transpose(1, 0, 2)
    )
    wo = np.stack(
        [np.ascontiguousarray(W_out[:, h * DH : (h + 1) * DH].T) for h in range(4)]
    )
    shared = {
        "wq": wq, "wk": wk, "wv": wv, "wo": wo,
        "bias": np.ascontiguousarray(b_out.reshape(3, 128)), "temp": temp,
    }
    maps = []
    for core in range(NCORES):
        b, half = core // 2, core % 2
        xs = x8[b, half * NL : (half + 1) * NL]  # (NL, C)
        xt8 = np.ascontiguousarray(
            xs.reshape(NK, 2, 128, C).transpose(0, 2, 1, 3)
        )
        xcm = np.ascontiguousarray(
            xcm_full[b, :, half * NL : (half + 1) * NL]
        ).reshape(3, 128, NL)
        maps.append(dict(shared, xt8=xt8, xcm=xcm))
    return maps


def kernel(**inputs):
    nc = get_nc(1)
    in_maps = prep_in_maps(
        inputs["x"],
        inputs["W_qkv"],
        inputs["temperature_ch"],
        inputs["W_out"],
        inputs["b_out"],
    )
    res = run_bass_kernel_spmd(nc, in_maps, core_ids=list(range(NCORES)))
    out = np.empty((B, NFULL, C), np.float32)
    for core in range(NCORES):
        b, half = core // 2, core % 2
        r = res.results[core]["out"]  # (3, 128, NL) bf16, channel-major
        out[b, half * NL : (half + 1) * NL, :] = (
            r.reshape(C, NL).T.astype(np.float32)
        )
    return out
